# revision 17
# baseline (speedup 1.0000x reference)
"""BitLinear (ternary-quantized linear) Trainium2 kernel — fp8 DoubleRow.

out = (x @ clip(round(W / scale), -1, 1).T) * scale,  scale = mean(|W|) + 1e-5

Sharding: tensor-parallel over out_features (11008 = 8 * 1376). Every core
receives the full activation x plus its own transposed weight shard
[4096, 1376]; the host concatenates the per-core [8192, 1376] slices.

Precision: the ternary weights {-1,0,1} are exact in fp8 (e4m3), and the
matmul runs with perf_mode=DoubleRow (two fp8 weights per PE cell,
256-deep contraction per pass) at 2x the bf16 MAC rate. x is quantized
to e4m3 on the host (RNE; TRN FP8_EXP4 and ml_dtypes float8_e4m3 agree
bit-for-bit in the range used here). Host-exact simulation of the
resulting error vs the fp32 reference on these inputs: rel 1.62e-2 of
absmax (tolerance 2e-2); device matches the simulation to the printed
digit. Weight quantization is exact and done on the host: fp32 scale,
exact fp32 threshold B where RNE of w/scale crosses +-0.5,
Wq = (w >= B) - (w <= -B), encoded directly as e4m3 bytes.

Schedule: W lives in 16 fine tiles [128,2,1376] and block 0 of x in 16
fine tiles [128,2,512], issued interleaved in k-consumption order so the
first matmul only waits for ~480KB of DMA. Eight warm-up matmuls on
memset tiles keep the PE busy (and the HAM clock-gate open) while the
first real tiles land. The last m-tile runs its n-chains serialized so
two of its three drains overlap compute.
"""

import os
import numpy as np

B_, S_, D_, O_ = 4, 2048, 4096, 11008
NCORES = 8
FO = O_ // NCORES            # 1376 out-features per core
TOK = B_ * S_                # 8192 tokens
KT2 = D_ // 256              # 16 k-tiles of 256 (DoubleRow contraction)
MB = 512                     # tokens per block
NT = [(0, 512), (512, 512), (1024, 352)]   # n-tile split of FO
CKS = 8                      # 128-row k-slabs per regular x chunk tile
NCK = (D_ // 128) // CKS     # 4 chunks per block
EPS = 1e-5
NWARM = 8                    # PE warm-up matmuls

_cache = {}


def _build_program(n_tokens=TOK):
    import concourse.bacc as bacc
    import concourse.mybir as mybir
    from concourse import tile

    f32 = mybir.dt.float32
    f8 = mybir.dt.float8e4
    Act = mybir.ActivationFunctionType
    DR = mybir.MatmulPerfMode.DoubleRow

    nmb = n_tokens // MB
    mtpb = MB // 128             # m-tiles per block

    nc = bacc.Bacc("TRN2", target_bir_lowering=False, debug=False,
                   num_devices=NCORES)

    xt_d = nc.dram_tensor("xt", [D_, n_tokens], f8, kind="ExternalInput")
    wt_d = nc.dram_tensor("wt", [D_, FO], f8, kind="ExternalInput")
    par_d = nc.dram_tensor("params", [128, 4], f32, kind="ExternalInput")
    out_d = nc.dram_tensor("out", [n_tokens, FO], f32, kind="ExternalOutput")

    with tile.TileContext(nc) as tc:
        from contextlib import ExitStack
        with ExitStack() as ctx:
            const = ctx.enter_context(tc.tile_pool(name="const", bufs=1))
            wqpool = ctx.enter_context(tc.tile_pool(name="wq", bufs=1))
            xfine = ctx.enter_context(tc.tile_pool(name="xfine", bufs=1))
            xblk = ctx.enter_context(tc.tile_pool(name="xblk", bufs=2))
            outp = ctx.enter_context(tc.tile_pool(name="outp", bufs=2))
            psum = ctx.enter_context(tc.tile_pool(name="psum", bufs=2,
                                                  space="PSUM"))

            pt = const.tile([128, 4], f32)
            nc.sync.dma_start(pt[:], par_d[:])
            scale_ap = pt[:, 0:1]

            # --- prologue DMAs in k-consumption order: (W_t, xf_t) pairs ---
            wch, xf0 = [], []
            for t in range(KT2):
                wt = wqpool.tile([128, 2, FO], f8, tag=f"w{t}", name=f"w{t}")
                nc.sync.dma_start(
                    wt[:], wt_d[t * 256:(t + 1) * 256, :]
                    .rearrange("(kt p) n -> p kt n", p=128))
                wch.append(wt)
                xt = xfine.tile([128, 2, MB], f8, tag=f"xf{t}", name=f"xf{t}")
                nc.sync.dma_start(
                    xt[:], xt_d[t * 256:(t + 1) * 256, 0:MB]
                    .rearrange("(kt p) m -> p kt m", p=128))
                xf0.append(xt)

            # --- PE warm-up: matmuls on memset tiles, no DMA deps ---
            wlhs = const.tile([128, 2, 128], f8)
            wrhs = const.tile([128, 2, 512], f8)
            nc.vector.memset(wlhs[:], 0.0)
            nc.vector.memset(wrhs[:], 0.0)
            for i in range(NWARM):
                pw = psum.tile([128, 512], f32, tag="ps3", name="pw")
                nc.tensor.matmul(pw[:], wlhs[:], wrhs[:],
                                 start=True, stop=True, perf_mode=DR)

            def wslice(t, j):
                n0, nw = NT[j]
                return wch[t][:, :, n0:n0 + nw]

            def load_x_block(mb):
                tiles = [xblk.tile([128, CKS, MB], f8, tag=f"xh{c}",
                                   name=f"xh{c}") for c in range(NCK)]
                for c in range(NCK):
                    src = xt_d[c * CKS * 128:(c + 1) * CKS * 128,
                               mb * MB:(mb + 1) * MB]
                    nc.sync.dma_start(
                        tiles[c][:], src.rearrange("(kt p) m -> p kt m",
                                                   p=128))
                return tiles

            def lh(xc, t, mt):
                if xc is None:                       # block 0: fine tiles
                    return xf0[t][:, :, mt * 128:(mt + 1) * 128]
                s = (2 * t) % CKS
                return xc[t // 4][:, s:s + 2, mt * 128:(mt + 1) * 128]

            def drain(ps, j, row):
                n0, nw = NT[j]
                o = outp.tile([128, nw], f32, tag=f"o{j}", name=f"o{j}")
                nc.scalar.activation(o[:], ps[:, :nw], Act.Copy,
                                     scale=scale_ap)
                nc.sync.dma_start(out_d[row:row + 128, n0:n0 + nw], o[:])

            # --- main loop: DoubleRow fp8 matmuls, 256-deep k-tiles ---
            cnt = 0
            for mb in range(nmb):
                xhi = None if mb == 0 else load_x_block(mb)
                for mt in range(mtpb):
                    row = (mb * mtpb + mt) * 128
                    last = (mb == nmb - 1) and (mt == mtpb - 1)
                    if last:
                        # serialize chains, shrinking toward the end, so
                        # drains and out-DMAs overlap the remaining compute
                        for n0, nw in [(0, 512), (512, 512),
                                       (1024, 176), (1200, 176)]:
                            ps = psum.tile([128, 512], f32,
                                           tag=f"ps{cnt % 4}", name="psl")
                            cnt += 1
                            for t in range(KT2):
                                nc.tensor.matmul(
                                    ps[:, :nw], lh(xhi, t, mt),
                                    wch[t][:, :, n0:n0 + nw],
                                    start=(t == 0), stop=(t == KT2 - 1),
                                    perf_mode=DR)
                            o = outp.tile([128, nw], f32, tag="ol",
                                          name="ol")
                            nc.scalar.activation(o[:], ps[:, :nw], Act.Copy,
                                                 scale=scale_ap)
                            nc.sync.dma_start(
                                out_d[row:row + 128, n0:n0 + nw], o[:])
                        continue
                    pss = []
                    for j in range(len(NT)):
                        ps = psum.tile([128, 512], f32, tag=f"ps{cnt % 4}",
                                       name=f"ps{j}")
                        cnt += 1
                        pss.append(ps)
                    for t in range(KT2):
                        xa = lh(xhi, t, mt)
                        for j in range(len(NT)):
                            nw = NT[j][1]
                            nc.tensor.matmul(
                                pss[j][:, :nw], xa, wslice(t, j),
                                start=(t == 0), stop=(t == KT2 - 1),
                                perf_mode=DR)
                    for j in range(len(NT)):
                        drain(pss[j], j, row)

    nc.compile()
    return nc


def _get_program(n_tokens=TOK):
    if n_tokens not in _cache:
        _cache[n_tokens] = _build_program(n_tokens)
    return _cache[n_tokens]


def _exact_threshold(scale):
    """Smallest fp32 v with fp32(v/scale) > 0.5 (RNE boundary of
    clip(round(w/scale)): w maps to +-1 iff w/scale rounds past +-0.5)."""
    scale = np.float32(scale)
    half = np.float32(0.5)
    v = np.float32(half * scale)
    while np.float32(v / scale) > half:
        v = np.nextafter(v, np.float32(0), dtype=np.float32)
    while not (np.float32(v / scale) > half):
        v = np.nextafter(v, np.float32(np.inf), dtype=np.float32)
    return v


LAST_RESULTS = None  # BassKernelResults of the most recent run (for test.py)


def kernel(x, weight):
    import ml_dtypes
    from concourse.bass_utils import run_bass_kernel_spmd

    x = np.asarray(x, dtype=np.float32)
    weight = np.asarray(weight, dtype=np.float32)
    n_tokens = x.shape[0] * x.shape[1]

    # scalar scale: fp32 mean(|W|) + eps, correctly rounded via an f64
    # accumulator (bit-matches jnp's fp32 mean on this input).
    scale = np.float32(np.float32(np.mean(np.abs(weight), dtype=np.float64))
                       + np.float32(EPS))
    bexact = _exact_threshold(scale)

    params = np.zeros((128, 4), np.float32)
    params[:, 0] = scale

    # exact ternary weights as e4m3 bytes: -1 -> 0xB8, 0 -> 0x00, 1 -> 0x38
    wq = ((weight >= bexact).astype(np.int8)
          - (weight <= -bexact).astype(np.int8))
    enc = np.array([0xB8, 0x00, 0x38], dtype=np.uint8)
    wq8 = enc[(wq + 1).astype(np.uint8)].view(ml_dtypes.float8_e4m3)

    # pre-transposed e4m3 activations [4096, n_tokens]
    xt = np.ascontiguousarray(
        x.reshape(n_tokens, D_).T).astype(ml_dtypes.float8_e4m3)
    in_maps = []
    for c in range(NCORES):
        wtc = np.ascontiguousarray(wq8[c * FO:(c + 1) * FO, :].T)
        in_maps.append({"xt": xt, "wt": wtc, "params": params})

    nc = _get_program(n_tokens)
    trace = bool(int(os.environ.get("KERNEL_TRACE", "0")))
    res = run_bass_kernel_spmd(nc, in_maps, list(range(NCORES)), trace=trace)
    global LAST_RESULTS
    LAST_RESULTS = res

    out = np.concatenate([res.results[c]["out"] for c in range(NCORES)],
                         axis=1)
    return out.reshape(x.shape[0], x.shape[1], O_)


# revision 18
# speedup vs baseline: 1.0014x; 1.0014x over previous
"""BitLinear (ternary-quantized linear) Trainium2 kernel — fp8 DoubleRow.

out = (x @ clip(round(W / scale), -1, 1).T) * scale,  scale = mean(|W|) + 1e-5

Sharding: tensor-parallel over out_features (11008 = 8 * 1376). Every core
receives the full activation x plus its own transposed weight shard
[4096, 1376]; the host concatenates the per-core [8192, 1376] slices.

Precision: the ternary weights {-1,0,1} are exact in fp8 (e4m3), and the
matmul runs with perf_mode=DoubleRow (two fp8 weights per PE cell,
256-deep contraction per pass) at 2x the bf16 MAC rate. x is quantized
to e4m3 on the host (RNE; TRN FP8_EXP4 and ml_dtypes float8_e4m3 agree
bit-for-bit in the range used here). Host-exact simulation of the
resulting error vs the fp32 reference on these inputs: rel 1.62e-2 of
absmax (tolerance 2e-2); device matches the simulation to the printed
digit. Weight quantization is exact and done on the host: fp32 scale,
exact fp32 threshold B where RNE of w/scale crosses +-0.5,
Wq = (w >= B) - (w <= -B), encoded directly as e4m3 bytes.

Schedule: W lives in 16 fine tiles [128,2,1376] and block 0 of x in 16
fine tiles [128,2,512], issued interleaved in k-consumption order so the
first matmul only waits for ~480KB of DMA. Eight warm-up matmuls on
memset tiles keep the PE busy (and the HAM clock-gate open) while the
first real tiles land. The last m-tile runs its n-chains serialized so
two of its three drains overlap compute.
"""

import os
import numpy as np

B_, S_, D_, O_ = 4, 2048, 4096, 11008
NCORES = 8
FO = O_ // NCORES            # 1376 out-features per core
TOK = B_ * S_                # 8192 tokens
KT2 = D_ // 256              # 16 k-tiles of 256 (DoubleRow contraction)
MB = 512                     # tokens per block
NT = [(0, 512), (512, 512), (1024, 352)]   # n-tile split of FO
CKS = 8                      # 128-row k-slabs per regular x chunk tile
NCK = (D_ // 128) // CKS     # 4 chunks per block
EPS = 1e-5
NWARM = 8                    # PE warm-up matmuls

_cache = {}


def _build_program(n_tokens=TOK):
    import concourse.bacc as bacc
    import concourse.mybir as mybir
    from concourse import tile

    f32 = mybir.dt.float32
    f8 = mybir.dt.float8e4
    Act = mybir.ActivationFunctionType
    DR = mybir.MatmulPerfMode.DoubleRow

    nmb = n_tokens // MB
    mtpb = MB // 128             # m-tiles per block

    nc = bacc.Bacc("TRN2", target_bir_lowering=False, debug=False,
                   num_devices=NCORES)

    xt_d = nc.dram_tensor("xt", [D_, n_tokens], f8, kind="ExternalInput")
    wt_d = nc.dram_tensor("wt", [D_, FO], f8, kind="ExternalInput")
    par_d = nc.dram_tensor("params", [128, 4], f32, kind="ExternalInput")
    out_d = nc.dram_tensor("out", [n_tokens, FO], f32, kind="ExternalOutput")

    with tile.TileContext(nc) as tc:
        from contextlib import ExitStack
        with ExitStack() as ctx:
            const = ctx.enter_context(tc.tile_pool(name="const", bufs=1))
            wqpool = ctx.enter_context(tc.tile_pool(name="wq", bufs=1))
            xfine = ctx.enter_context(tc.tile_pool(name="xfine", bufs=1))
            xblk = ctx.enter_context(tc.tile_pool(name="xblk", bufs=2))
            outp = ctx.enter_context(tc.tile_pool(name="outp", bufs=2))
            psum = ctx.enter_context(tc.tile_pool(name="psum", bufs=2,
                                                  space="PSUM"))

            pt = const.tile([128, 4], f32)
            nc.sync.dma_start(pt[:], par_d[:])
            scale_ap = pt[:, 0:1]

            # --- prologue DMAs in k-consumption order: (W_t, xf_t) pairs ---
            wch, xf0 = [], []
            for t in range(KT2):
                wt = wqpool.tile([128, 2, FO], f8, tag=f"w{t}", name=f"w{t}")
                nc.sync.dma_start(
                    wt[:], wt_d[t * 256:(t + 1) * 256, :]
                    .rearrange("(kt p) n -> p kt n", p=128))
                wch.append(wt)
                xt = xfine.tile([128, 2, MB], f8, tag=f"xf{t}", name=f"xf{t}")
                nc.sync.dma_start(
                    xt[:], xt_d[t * 256:(t + 1) * 256, 0:MB]
                    .rearrange("(kt p) m -> p kt m", p=128))
                xf0.append(xt)

            # --- PE warm-up: matmuls on memset tiles, no DMA deps ---
            wlhs = const.tile([128, 2, 128], f8)
            wrhs = const.tile([128, 2, 512], f8)
            nc.vector.memset(wlhs[:], 0.0)
            nc.vector.memset(wrhs[:], 0.0)
            for i in range(NWARM):
                pw = psum.tile([128, 512], f32, tag="ps3", name="pw")
                nc.tensor.matmul(pw[:], wlhs[:], wrhs[:],
                                 start=True, stop=True, perf_mode=DR)

            def wslice(t, j):
                n0, nw = NT[j]
                return wch[t][:, :, n0:n0 + nw]

            def load_x_block(mb):
                tiles = [xblk.tile([128, CKS, MB], f8, tag=f"xh{c}",
                                   name=f"xh{c}") for c in range(NCK)]
                for c in range(NCK):
                    src = xt_d[c * CKS * 128:(c + 1) * CKS * 128,
                               mb * MB:(mb + 1) * MB]
                    nc.sync.dma_start(
                        tiles[c][:], src.rearrange("(kt p) m -> p kt m",
                                                   p=128))
                return tiles

            def lh(xc, t, mt):
                if xc is None:                       # block 0: fine tiles
                    return xf0[t][:, :, mt * 128:(mt + 1) * 128]
                s = (2 * t) % CKS
                return xc[t // 4][:, s:s + 2, mt * 128:(mt + 1) * 128]

            def drain(ps, j, row):
                n0, nw = NT[j]
                o = outp.tile([128, nw], f32, tag=f"o{j}", name=f"o{j}")
                nc.scalar.activation(o[:], ps[:, :nw], Act.Copy,
                                     scale=scale_ap)
                nc.sync.dma_start(out_d[row:row + 128, n0:n0 + nw], o[:])

            # --- main loop: DoubleRow fp8 matmuls, 256-deep k-tiles ---
            cnt = 0
            for mb in range(nmb):
                xhi = None if mb == 0 else load_x_block(mb)
                for mt in range(mtpb):
                    row = (mb * mtpb + mt) * 128
                    last = (mb == nmb - 1) and (mt == mtpb - 1)
                    if last:
                        # serialize chains so early drains overlap compute
                        for j in range(len(NT)):
                            nw = NT[j][1]
                            ps = psum.tile([128, 512], f32,
                                           tag=f"ps{cnt % 4}", name=f"ps{j}")
                            cnt += 1
                            for t in range(KT2):
                                nc.tensor.matmul(
                                    ps[:, :nw], lh(xhi, t, mt), wslice(t, j),
                                    start=(t == 0), stop=(t == KT2 - 1),
                                    perf_mode=DR)
                            drain(ps, j, row)
                        continue
                    pss = []
                    for j in range(len(NT)):
                        ps = psum.tile([128, 512], f32, tag=f"ps{cnt % 4}",
                                       name=f"ps{j}")
                        cnt += 1
                        pss.append(ps)
                    for t in range(KT2):
                        xa = lh(xhi, t, mt)
                        for j in range(len(NT)):
                            nw = NT[j][1]
                            nc.tensor.matmul(
                                pss[j][:, :nw], xa, wslice(t, j),
                                start=(t == 0), stop=(t == KT2 - 1),
                                perf_mode=DR)
                    for j in range(len(NT)):
                        drain(pss[j], j, row)

    nc.compile()
    return nc


def _get_program(n_tokens=TOK):
    if n_tokens not in _cache:
        _cache[n_tokens] = _build_program(n_tokens)
    return _cache[n_tokens]


def _exact_threshold(scale):
    """Smallest fp32 v with fp32(v/scale) > 0.5 (RNE boundary of
    clip(round(w/scale)): w maps to +-1 iff w/scale rounds past +-0.5)."""
    scale = np.float32(scale)
    half = np.float32(0.5)
    v = np.float32(half * scale)
    while np.float32(v / scale) > half:
        v = np.nextafter(v, np.float32(0), dtype=np.float32)
    while not (np.float32(v / scale) > half):
        v = np.nextafter(v, np.float32(np.inf), dtype=np.float32)
    return v


LAST_RESULTS = None  # BassKernelResults of the most recent run (for test.py)


def kernel(x, weight):
    import ml_dtypes
    from concourse.bass_utils import run_bass_kernel_spmd

    x = np.asarray(x, dtype=np.float32)
    weight = np.asarray(weight, dtype=np.float32)
    n_tokens = x.shape[0] * x.shape[1]

    # scalar scale: fp32 mean(|W|) + eps, correctly rounded via an f64
    # accumulator (bit-matches jnp's fp32 mean on this input).
    scale = np.float32(np.float32(np.mean(np.abs(weight), dtype=np.float64))
                       + np.float32(EPS))
    bexact = _exact_threshold(scale)

    params = np.zeros((128, 4), np.float32)
    params[:, 0] = scale

    # exact ternary weights as e4m3 bytes: -1 -> 0xB8, 0 -> 0x00, 1 -> 0x38
    wq = ((weight >= bexact).astype(np.int8)
          - (weight <= -bexact).astype(np.int8))
    enc = np.array([0xB8, 0x00, 0x38], dtype=np.uint8)
    wq8 = enc[(wq + 1).astype(np.uint8)].view(ml_dtypes.float8_e4m3)

    # pre-transposed e4m3 activations [4096, n_tokens]
    xt = np.ascontiguousarray(
        x.reshape(n_tokens, D_).T).astype(ml_dtypes.float8_e4m3)
    in_maps = []
    for c in range(NCORES):
        wtc = np.ascontiguousarray(wq8[c * FO:(c + 1) * FO, :].T)
        in_maps.append({"xt": xt, "wt": wtc, "params": params})

    nc = _get_program(n_tokens)
    trace = bool(int(os.environ.get("KERNEL_TRACE", "0")))
    res = run_bass_kernel_spmd(nc, in_maps, list(range(NCORES)), trace=trace)
    global LAST_RESULTS
    LAST_RESULTS = res

    out = np.concatenate([res.results[c]["out"] for c in range(NCORES)],
                         axis=1)
    return out.reshape(x.shape[0], x.shape[1], O_)
